# revision 7
# baseline (speedup 1.0000x reference)
"""Trainium2 Bass kernel v2 for nn_EdgeUpdate (gnn_message_passing).

reference math:
    atom_scalars = atom_features @ W_lin                       # [N, H]
    edge_in = concat([s[dst], s[src], edge_features], -1)      # [E, 3H]
    h = relu(edge_in @ W1 + b1); h = relu(h @ W2 + b2); h = h @ W3 + b3
    out = layernorm(edge_features + h) * gamma + beta          # [E, H]

v2 strategy (vs v1's gpsimd ap_gather, which paid a full 32768-elem
table scan per 1024-edge gather = 5.7ms of Pool time):
  - atom-scalar table kept in SBUF as bf16 rank-stripes (atom a ->
    partition a%128, stripe a//128, 256B/atom; 62.5KB/partition), built
    transpose-free with stationary=atomT-block matmuls.
  - edge gathers via gpsimd.dma_gather(transpose=True, SBUF source):
    descriptor-gen on Pool (~0.34ns/idx), data moved by the 16 SDMA
    engines, output lands directly in [feature, edge] layout.
  - edge_features arrive host-transposed/bf16 (efT) and pair-permuted;
    MLP runs in T layout; residual add in T; one bf16 PE transpose back;
    LayerNorm via bn_stats in [edge, H] layout; bf16 output written as
    512B paired-row descriptors. All matmuls bf16 (fp32 is 4 cyc/col).
Per-core: 64000 edges, 10 batches x (2 dma_gathers + efT load + out
store) + 125 supertiles of 512 edges (12x512+256 per batch).
"""

import collections
import sys
import numpy as np

sys.path.insert(0, "/opt/trn_rl_repo")

import ml_dtypes  # noqa: E402

import concourse.bacc as bacc  # noqa: E402
import concourse.tile as tile  # noqa: E402
import concourse.mybir as mybir  # noqa: E402
from concourse.masks import make_identity  # noqa: E402

N_CORES = 8
N_ATOM = 32000
E_EDGE = 512000
D_IN = 256
H = 128
P = 128
ESH = E_EDGE // N_CORES          # 64000 edges per core
NB = 2560                        # edges per gather batch (multiple of 256)
NBATCH = ESH // NB               # 25
GC = 896                         # idxs per dma_gather call (desc-ring cap)
SUP = 512                        # edges per supertile (= PSUM bank)
RANKS = N_ATOM // P              # 250 table stripes
LN_EPS = 1e-5

F32 = mybir.dt.float32
BF16 = mybir.dt.bfloat16
I16 = mybir.dt.int16
AF = mybir.ActivationFunctionType
ALU = mybir.AluOpType

_CACHE = {}


def _build(trivial_affine: bool, zero_bias: bool, ablate: frozenset = frozenset()):
    import os
    if not ablate and os.environ.get("K_ABLATE"):
        ablate = frozenset(os.environ["K_ABLATE"].split(","))
    nc = bacc.Bacc("TRN2", target_bir_lowering=False, debug=False,
                   enable_asserts=False, num_devices=N_CORES,
                   dynamic_dma_scratch_size=65536, num_swdge_queues=4)

    atomT_d = nc.dram_tensor("atomT", [2, P, N_ATOM], BF16, kind="ExternalInput")
    efT_d = nc.dram_tensor("efT", [P, ESH], BF16, kind="ExternalInput")
    idxd_d = nc.dram_tensor("idx_dst", [P, ESH // 16], I16, kind="ExternalInput")
    idxs_d = nc.dram_tensor("idx_src", [P, ESH // 16], I16, kind="ExternalInput")
    wl_d = nc.dram_tensor("wl16", [2, P, H], BF16, kind="ExternalInput")
    w1_d = nc.dram_tensor("w1t16", [3, P, H], BF16, kind="ExternalInput")
    w2_d = nc.dram_tensor("w2t16", [P, H], BF16, kind="ExternalInput")
    w3_d = nc.dram_tensor("w3t16", [P, H], BF16, kind="ExternalInput")
    if not zero_bias:
        b1_d = nc.dram_tensor("b1", [P, 1], F32, kind="ExternalInput")
        b2_d = nc.dram_tensor("b2", [P, 1], F32, kind="ExternalInput")
        b3_d = nc.dram_tensor("b3", [P, 1], F32, kind="ExternalInput")
    if not trivial_affine:
        gam_d = nc.dram_tensor("gam", [P, H], F32, kind="ExternalInput")
        bet_d = nc.dram_tensor("bet", [P, H], F32, kind="ExternalInput")
    out_d = nc.dram_tensor("out", [ESH, H], BF16, kind="ExternalOutput")

    with tile.TileContext(nc) as tc:
        with tc.tile_pool(name="const", bufs=1) as const:
            # --- constants ---------------------------------------------------
            wl0 = const.tile([P, H], BF16)
            nc.sync.dma_start(out=wl0[:], in_=wl_d[0])
            wl1 = const.tile([P, H], BF16)
            nc.sync.dma_start(out=wl1[:], in_=wl_d[1])
            w1a = const.tile([P, H], BF16)
            nc.sync.dma_start(out=w1a[:], in_=w1_d[0])
            w1b = const.tile([P, H], BF16)
            nc.sync.dma_start(out=w1b[:], in_=w1_d[1])
            w1c = const.tile([P, H], BF16)
            nc.sync.dma_start(out=w1c[:], in_=w1_d[2])
            w2 = const.tile([P, H], BF16)
            nc.sync.dma_start(out=w2[:], in_=w2_d[:])
            w3 = const.tile([P, H], BF16)
            nc.sync.dma_start(out=w3[:], in_=w3_d[:])
            if not zero_bias:
                b1 = const.tile([P, 1], F32)
                nc.sync.dma_start(out=b1[:], in_=b1_d[:])
                b2 = const.tile([P, 1], F32)
                nc.sync.dma_start(out=b2[:], in_=b2_d[:])
                b3 = const.tile([P, 1], F32)
                nc.sync.dma_start(out=b3[:], in_=b3_d[:])
            else:
                b1 = b2 = b3 = None
            if not trivial_affine:
                gam = const.tile([P, H], F32)
                nc.sync.dma_start(out=gam[:], in_=gam_d[:])
                bet = const.tile([P, H], F32)
                nc.sync.dma_start(out=bet[:], in_=bet_d[:])
            else:
                gam = bet = None
            ident = const.tile([P, P], BF16)
            make_identity(nc, ident[:])
            eps_t = const.tile([P, 1], F32)
            nc.vector.memset(eps_t[:], LN_EPS)
            idxd = const.tile([P, ESH // 16], I16)
            nc.sync.dma_start(out=idxd[:], in_=idxd_d[:])
            idxs = const.tile([P, ESH // 16], I16)
            nc.sync.dma_start(out=idxs[:], in_=idxs_d[:])
            # atom table, rank-stripe layout: atom a -> partition a%128,
            # free bytes [ (a//128)*256, +256 ) as 128 bf16 features.
            table = const.tile([P, RANKS, H], BF16)        # 62.5KB/partition

            # --- table build: s = A @ W_lin, written block-transposed -------
            # stationary lhsT = atomT half-block [128 infeat, 128 atoms]
            # moving rhs = W_lin half [128 infeat, 128 outfeat]
            # out = [128 atoms, 128 feats] = exactly one table stripe.
            ACH = 6400                                     # atoms per dma chunk
            assert N_ATOM % ACH == 0 and ACH % P == 0
            with tc.tile_pool(name="bld", bufs=2) as bld, \
                 tc.tile_pool(name="bldps", bufs=4, space="PSUM") as bldps:
                for ci in range(N_ATOM // ACH):
                    off = ci * ACH
                    a0 = bld.tile([P, ACH], BF16, tag="a0")
                    nc.sync.dma_start(out=a0[:], in_=atomT_d[0, :, off:off + ACH])
                    a1 = bld.tile([P, ACH], BF16, tag="a1")
                    nc.sync.dma_start(out=a1[:], in_=atomT_d[1, :, off:off + ACH])
                    for g in range((ACH + 511) // 512):    # 4 blocks per psum bank
                        nblk = min(4, (ACH - g * 512) // P)
                        ps = bldps.tile([P, 4, P], F32, space="PSUM", tag="bps")
                        for b in range(nblk):
                            s = g * 512 + b * P
                            nc.tensor.matmul(out=ps[:, b], lhsT=a0[:, s:s + P],
                                             rhs=wl0[:], start=True, stop=False)
                            nc.tensor.matmul(out=ps[:, b], lhsT=a1[:, s:s + P],
                                             rhs=wl1[:], start=False, stop=True)
                        st = (off + g * 512) // P          # first stripe idx
                        nc.vector.tensor_copy(
                            table[:, st:st + nblk].rearrange("p r f -> p (r f)"),
                            ps[:, :nblk].rearrange("p b f -> p (b f)"))

            # --- main loop ---------------------------------------------------
            tiles = [(i * SUP, SUP) for i in range(NB // SUP)]
            if NB % SUP:
                tiles.append(((NB // SUP) * SUP, NB % SUP))
            with tc.tile_pool(name="gat", bufs=2) as gat, \
                 tc.tile_pool(name="io", bufs=2) as io, \
                 tc.tile_pool(name="mid", bufs=4) as mid, \
                 tc.tile_pool(name="stat", bufs=8) as stat, \
                 tc.tile_pool(name="pmm", bufs=5, space="PSUM") as pmm, \
                 tc.tile_pool(name="ptr", bufs=3, space="PSUM") as ptr:
                def emit_batch(bi):
                    """Issue gathers + loads for batch bi; returns refs."""
                    e0 = bi * NB
                    i0 = bi * (NB // 16)
                    gd = gat.tile([P, 1, NB], BF16, tag="gd")
                    gs = gat.tile([P, 1, NB], BF16, tag="gs")
                    if "gather" in ablate:
                        nc.vector.memset(gd[:], 0.25)
                        nc.vector.memset(gs[:], 0.25)
                    else:
                        # chunk: the SWDGE descriptor ring (16KB/16=1024
                        # descs) caps a single dma_gather; >512 idxs hangs
                        # the exec unit on HW.
                        qrr = 0
                        for q0, (g, idxt) in enumerate(((gd, idxd), (gs, idxs))):
                            for off in range(0, NB, GC):
                                n = min(GC, NB - off)
                                q = qrr % 4
                                qrr += 1
                                nc.gpsimd.dma_gather(
                                    g[:, :, off:off + n],
                                    table[:].rearrange("p r f -> p (r f)"),
                                    idxt[:, i0 + off // 16:i0 + (off + n) // 16],
                                    n, n, H,
                                    transpose=True, sbuf_tokens_per_rank=P,
                                    sbuf_free_dim_per_rank=2 * H,
                                    queue_num=q)
                    eft = io.tile([P, NB], BF16, tag="eft")
                    nc.sync.dma_start(out=eft[:], in_=efT_d[:, e0:e0 + NB])
                    outb = io.tile([P, NB // 256, 2, P], BF16, tag="outb")
                    return dict(gd=gd, gs=gs, eft=eft, outb=outb, e0=e0)

                def mm_stage(B, toff, tw):
                    """L1..L3 matmuls + relus -> ps3 (PSUM)."""
                    ps1 = pmm.tile([P, SUP], F32, space="PSUM", tag="mm")
                    nc.tensor.matmul(out=ps1[:, :tw], lhsT=w1a[:],
                                     rhs=B["gd"][:, 0, toff:toff + tw],
                                     start=True, stop=False)
                    nc.tensor.matmul(out=ps1[:, :tw], lhsT=w1b[:],
                                     rhs=B["gs"][:, 0, toff:toff + tw],
                                     start=False, stop=False)
                    nc.tensor.matmul(out=ps1[:, :tw], lhsT=w1c[:],
                                     rhs=B["eft"][:, toff:toff + tw],
                                     start=False, stop=True)
                    h1 = mid.tile([P, SUP], BF16, tag="h1")
                    nc.scalar.activation(h1[:, :tw], ps1[:, :tw], AF.Relu,
                                         bias=b1[:, 0:1] if b1 else 0.0)
                    ps2 = pmm.tile([P, SUP], F32, space="PSUM", tag="mm")
                    nc.tensor.matmul(out=ps2[:, :tw], lhsT=w2[:],
                                     rhs=h1[:, :tw], start=True, stop=True)
                    h2 = mid.tile([P, SUP], BF16, tag="h2")
                    nc.scalar.activation(h2[:, :tw], ps2[:, :tw], AF.Relu,
                                         bias=b2[:, 0:1] if b2 else 0.0)
                    ps3 = pmm.tile([P, SUP], F32, space="PSUM", tag="mm")
                    nc.tensor.matmul(out=ps3[:, :tw], lhsT=w3[:],
                                     rhs=h2[:, :tw], start=True, stop=True)
                    return ps3

                def xt_stage(B, toff, tw, ps3):
                    """residual in T layout: xT = (ps3 + b3) + efT  (DVE)."""
                    xT = mid.tile([P, SUP], BF16, tag="xT")
                    if zero_bias:
                        nc.vector.tensor_tensor(
                            out=xT[:, :tw], in0=ps3[:, :tw],
                            in1=B["eft"][:, toff:toff + tw], op=ALU.add)
                    else:
                        h3s = mid.tile([P, SUP], BF16, tag="h3s")
                        nc.scalar.activation(h3s[:, :tw], ps3[:, :tw],
                                             AF.Identity, bias=b3[:, 0:1])
                        nc.vector.tensor_tensor(
                            out=xT[:, :tw], in0=h3s[:, :tw],
                            in1=B["eft"][:, toff:toff + tw], op=ALU.add)
                    return xT

                def tr_stage(tw, xT):
                    """transpose back (PE): block c = edges base+2p+(c%2)."""
                    nch = tw // P
                    xps = ptr.tile([P, 4, P], BF16, space="PSUM", tag="tr")
                    for c in range(nch):
                        nc.tensor.transpose(xps[:, c], xT[:, c * P:(c + 1) * P],
                                            ident[:])
                    return xps

                def ln_stage(B, toff, tw, xps):
                    """LN stats + normalize -> outb (DVE/Act)."""
                    nch = tw // P
                    outb = B["outb"]
                    bn = stat.tile([P, 4, 6], F32, tag="bn")
                    mv = stat.tile([P, 4, 2], F32, tag="mv")
                    for c in range(nch):
                        nc.vector.bn_stats(bn[:, c], xps[:, c])
                        nc.vector.bn_aggr(mv[:, c], bn[:, c])
                    std = stat.tile([P, 4], F32, tag="std")
                    nc.scalar.activation(std[:, :nch], mv[:, :nch, 1],
                                         AF.Sqrt, bias=eps_t[:, 0:1])
                    rstd = stat.tile([P, 4], F32, tag="rstd")
                    nc.vector.reciprocal(rstd[:, :nch], std[:, :nch])
                    nmr = stat.tile([P, 4], F32, tag="nmr")
                    nc.vector.tensor_tensor(out=nmr[:, :nch],
                                            in0=mv[:, :nch, 0],
                                            in1=rstd[:, :nch], op=ALU.mult)
                    nc.vector.tensor_scalar(out=nmr[:, :nch],
                                            in0=nmr[:, :nch], scalar1=-1.0,
                                            scalar2=None, op0=ALU.mult)
                    # apply: out = x*rstd + (-mean*rstd)  [+ affine]
                    for c in range(nch):
                        kk = toff // 256 + c // 2
                        j = c % 2
                        if c != 0:
                            nc.scalar.activation(outb[:, kk, j], xps[:, c],
                                                 AF.Identity,
                                                 bias=nmr[:, c:c + 1],
                                                 scale=rstd[:, c:c + 1])
                        else:
                            nc.vector.tensor_scalar(
                                out=outb[:, kk, j], in0=xps[:, c],
                                scalar1=rstd[:, c:c + 1],
                                scalar2=nmr[:, c:c + 1],
                                op0=ALU.mult, op1=ALU.add)
                        if not trivial_affine:
                            nc.vector.tensor_tensor(out=outb[:, kk, j],
                                                    in0=outb[:, kk, j],
                                                    in1=gam[:], op=ALU.mult)
                            nc.vector.tensor_tensor(out=outb[:, kk, j],
                                                    in0=outb[:, kk, j],
                                                    in1=bet[:], op=ALU.add)

                def out_stage(B):
                    nc.sync.dma_start(
                        out=out_d[B["e0"]:B["e0"] + NB, :].rearrange(
                            "(k p j) f -> p k (j f)", p=P, j=2),
                        in_=B["outb"][:])

                # two-deep software pipeline ordered so each engine's
                # in-order stream is ready-when-reached:
                #   mm(t) [PE/Act] ; xT(t-1) [DVE] ; tr(t-1) [PE] ;
                #   ln(t-2) [DVE/Act]
                work = [(bi, toff, tw) for bi in range(NBATCH)
                        for (toff, tw) in tiles]
                batches = {}
                s1 = s2 = None          # (bi, toff, tw, ps3) / (.., xps)
                ln_done = collections.Counter()
                ntiles = len(tiles)

                def run_ln(item):
                    bi, toff, tw, xps = item
                    ln_stage(batches[bi], toff, tw, xps)
                    ln_done[bi] += 1
                    if ln_done[bi] == ntiles:
                        out_stage(batches[bi])

                if "mlp" in ablate:
                    for bi in range(NBATCH):
                        B = emit_batch(bi)
                        nc.vector.tensor_copy(
                            B["outb"][:].rearrange("p k j f -> p (k j f)"),
                            B["gd"][:, 0].bitcast(BF16)[:, :NB])
                        out_stage(B)
                else:
                    for (bi, toff, tw) in work:
                        if bi not in batches:
                            batches[bi] = emit_batch(bi)
                        ps3 = mm_stage(batches[bi], toff, tw)
                        if s1 is not None:
                            b1_, t1, w1_, p3 = s1
                            xT = xt_stage(batches[b1_], t1, w1_, p3)
                            xps = tr_stage(w1_, xT)
                            if s2 is not None:
                                run_ln(s2)
                            s2 = (b1_, t1, w1_, xps)
                        s1 = (bi, toff, tw, ps3)
                    # drain
                    b1_, t1, w1_, p3 = s1
                    xT = xt_stage(batches[b1_], t1, w1_, p3)
                    xps = tr_stage(w1_, xT)
                    if s2 is not None:
                        run_ln(s2)
                    run_ln((b1_, t1, w1_, xps))

    nc.compile()
    return nc


def _make_runner(nc):
    """shard_map runner over 8 cores (no donation so it can be re-invoked)."""
    import jax
    from jax.sharding import Mesh, PartitionSpec
    from jax.experimental.shard_map import shard_map
    from concourse import bass2jax

    bass2jax.install_neuronx_cc_hook()

    partition_name = (nc.partition_id_tensor.name
                      if nc.partition_id_tensor else None)
    in_names, out_names, out_avals, zero_shapes = [], [], [], []
    for alloc in nc.m.functions[0].allocations:
        if not isinstance(alloc, mybir.MemoryLocationSet):
            continue
        name = alloc.memorylocations[0].name
        if alloc.kind == "ExternalInput":
            if name != partition_name:
                in_names.append(name)
        elif alloc.kind == "ExternalOutput":
            out_names.append(name)
            out_avals.append(jax.core.ShapedArray(
                tuple(alloc.tensor_shape), mybir.dt.np(alloc.dtype)))
            zero_shapes.append((tuple(alloc.tensor_shape), mybir.dt.np(alloc.dtype)))
    n_params = len(in_names)
    all_in_names = in_names + out_names
    if partition_name is not None:
        all_in_names = all_in_names + [partition_name]

    def _body(*args):
        operands = list(args)
        if partition_name is not None:
            operands.append(bass2jax.partition_id_tensor())
        outs = bass2jax._bass_exec_p.bind(
            *operands,
            out_avals=tuple(out_avals),
            in_names=tuple(all_in_names),
            out_names=tuple(out_names),
            lowering_input_output_aliases=(),
            sim_require_finite=True,
            sim_require_nnan=True,
            nc=nc,
        )
        return tuple(outs)

    devices = jax.devices()[:N_CORES]
    mesh = Mesh(np.asarray(devices), ("core",))
    nin = n_params + len(out_names)
    sharded = jax.jit(
        shard_map(_body, mesh=mesh,
                  in_specs=(PartitionSpec("core"),) * nin,
                  out_specs=(PartitionSpec("core"),) * len(out_names),
                  check_rep=False),
        keep_unused=True)
    return sharded, in_names, out_names, zero_shapes


def _pair_permute_idx(idx_flat: np.ndarray) -> np.ndarray:
    """Reorder edges within each 256-chunk to [evens, odds]."""
    return idx_flat.reshape(-1, P, 2).transpose(0, 2, 1).reshape(-1)


def _wrap_idx(idx_flat: np.ndarray) -> np.ndarray:
    """ap/dma_gather wrapped-index layout: idx[p, s] covers edge s*16 + p%16,
    replicated across the 8 gpsimd 16-partition core groups."""
    a = idx_flat.astype(np.int16).reshape(-1, 16).T        # [16, esh//16]
    return np.tile(a, (8, 1))                              # [128, esh//16]


def _prep(inputs):
    atom_features = np.asarray(inputs["atom_features"], dtype=np.float32)
    edge_features = np.asarray(inputs["edge_features"], dtype=np.float32)
    edge_index = np.asarray(inputs["edge_index"]).astype(np.int64)
    wlin = np.asarray(inputs["W_lin"], dtype=np.float32)
    w1 = np.asarray(inputs["W1"], dtype=np.float32)
    w2 = np.asarray(inputs["W2"], dtype=np.float32)
    w3 = np.asarray(inputs["W3"], dtype=np.float32)
    b1 = np.asarray(inputs["b1"], dtype=np.float32)
    b2 = np.asarray(inputs["b2"], dtype=np.float32)
    b3 = np.asarray(inputs["b3"], dtype=np.float32)
    gamma = np.asarray(inputs["gamma"], dtype=np.float32)
    beta = np.asarray(inputs["beta"], dtype=np.float32)

    trivial_affine = bool(np.all(gamma == 1.0) and np.all(beta == 0.0))
    zero_bias = bool(np.all(b1 == 0.0) and np.all(b2 == 0.0) and np.all(b3 == 0.0))

    bf = ml_dtypes.bfloat16
    at = atom_features.T.astype(bf)                        # [256, 32000]
    atomT = np.stack([at[:P], at[P:]])                     # [2, 128, 32000]

    shared = {
        "atomT": atomT,
        "wl16": wlin.astype(bf).reshape(2, P, H),
        "w1t16": w1.astype(bf).reshape(3, P, H),
        "w2t16": w2.astype(bf),
        "w3t16": w3.astype(bf),
    }
    if not zero_bias:
        shared["b1"] = b1.reshape(P, 1)
        shared["b2"] = b2.reshape(P, 1)
        shared["b3"] = b3.reshape(P, 1)
    if not trivial_affine:
        shared["gam"] = np.tile(gamma.reshape(1, H), (P, 1)).astype(np.float32)
        shared["bet"] = np.tile(beta.reshape(1, H), (P, 1)).astype(np.float32)

    in_maps = []
    for c in range(N_CORES):
        e0 = c * ESH
        ef = edge_features[e0:e0 + ESH]
        # pair-permute: within each 256-edge chunk -> [evens, odds]
        efp = ef.reshape(-1, P, 2, H).transpose(0, 2, 1, 3).reshape(ESH, H)
        m = dict(shared)
        m["efT"] = efp.T.astype(bf)                        # [128, 64000]
        m["idx_dst"] = _wrap_idx(_pair_permute_idx(edge_index[0, e0:e0 + ESH]))
        m["idx_src"] = _wrap_idx(_pair_permute_idx(edge_index[1, e0:e0 + ESH]))
        in_maps.append(m)
    return in_maps, trivial_affine, zero_bias


def _get_compiled(trivial_affine: bool, zero_bias: bool):
    key = ("k2", trivial_affine, zero_bias)
    if key not in _CACHE:
        nc = _build(trivial_affine, zero_bias)
        runner = _make_runner(nc)
        _CACHE[key] = (nc, runner)
    return _CACHE[key]


def _concat_inputs(in_maps, in_names, zero_shapes):
    concat_in = [
        np.concatenate([np.asarray(in_maps[c][n]) for c in range(N_CORES)], axis=0)
        for n in in_names
    ]
    concat_zero = [
        np.zeros((N_CORES * s[0], *s[1:]), dt) for (s, dt) in zero_shapes
    ]
    return concat_in, concat_zero


def kernel(**inputs) -> np.ndarray:
    in_maps, trivial_affine, zero_bias = _prep(inputs)
    _, (sharded, in_names, out_names, zero_shapes) = _get_compiled(
        trivial_affine, zero_bias)
    concat_in, concat_zero = _concat_inputs(in_maps, in_names, zero_shapes)
    outs = sharded(*concat_in, *concat_zero)
    oi = out_names.index("out")
    full = np.asarray(outs[oi]).reshape(N_CORES * ESH, H)
    return full.astype(np.float32)


def bench(inputs, reps: int = 10):
    """Returns (exec_times_seconds, results) using device-resident inputs."""
    import jax, time
    from jax.sharding import Mesh, PartitionSpec, NamedSharding
    in_maps, trivial_affine, zero_bias = _prep(inputs)
    _, (sharded, in_names, out_names, zero_shapes) = _get_compiled(
        trivial_affine, zero_bias)
    concat_in, concat_zero = _concat_inputs(in_maps, in_names, zero_shapes)
    devices = jax.devices()[:N_CORES]
    mesh = Mesh(np.asarray(devices), ("core",))
    sh = NamedSharding(mesh, PartitionSpec("core"))
    args = [jax.device_put(a, sh) for a in concat_in + concat_zero]
    outs = sharded(*args)  # warm-up + compile
    jax.block_until_ready(outs)
    times = []
    for _ in range(reps):
        t0 = time.perf_counter()
        outs = sharded(*args)
        jax.block_until_ready(outs)
        times.append(time.perf_counter() - t0)
    # pipelined dispatch: amortizes per-call host/tunnel overhead
    npipe = 30
    t0 = time.perf_counter()
    for _ in range(npipe):
        outs = sharded(*args)
    jax.block_until_ready(outs)
    pipe_per_call = (time.perf_counter() - t0) / npipe
    times.append(pipe_per_call)
    oi = out_names.index("out")
    full = np.asarray(outs[oi]).reshape(N_CORES * ESH, H).astype(np.float32)
    return times, full


# revision 11
# speedup vs baseline: 1.0564x; 1.0564x over previous
"""Trainium2 Bass kernel v2 for nn_EdgeUpdate (gnn_message_passing).

reference math:
    atom_scalars = atom_features @ W_lin                       # [N, H]
    edge_in = concat([s[dst], s[src], edge_features], -1)      # [E, 3H]
    h = relu(edge_in @ W1 + b1); h = relu(h @ W2 + b2); h = h @ W3 + b3
    out = layernorm(edge_features + h) * gamma + beta          # [E, H]

v2 strategy (vs v1's gpsimd ap_gather, which paid a full 32768-elem
table scan per 1024-edge gather = 5.7ms of Pool time):
  - atom-scalar table kept in SBUF as bf16 rank-stripes (atom a ->
    partition a%128, stripe a//128, 256B/atom; 62.5KB/partition), built
    transpose-free with stationary=atomT-block matmuls.
  - edge gathers via gpsimd.dma_gather(transpose=True, SBUF source):
    descriptor-gen on Pool (~0.34ns/idx), data moved by the 16 SDMA
    engines, output lands directly in [feature, edge] layout.
  - edge_features arrive host-transposed/bf16 (efT) and pair-permuted;
    MLP runs in T layout; residual add in T; one bf16 PE transpose back;
    LayerNorm via bn_stats in [edge, H] layout; bf16 output written as
    512B paired-row descriptors. All matmuls bf16 (fp32 is 4 cyc/col).
Per-core: 64000 edges, 25 batches x (chunked dma_gathers + efT load + out
store) + 125 supertiles of 512 edges (5x512 per batch).
"""

import collections
import sys
import numpy as np

sys.path.insert(0, "/opt/trn_rl_repo")

import ml_dtypes  # noqa: E402

import concourse.bacc as bacc  # noqa: E402
import concourse.tile as tile  # noqa: E402
import concourse.mybir as mybir  # noqa: E402
from concourse.masks import make_identity  # noqa: E402

N_CORES = 8
N_ATOM = 32000
E_EDGE = 512000
D_IN = 256
H = 128
P = 128
ESH = E_EDGE // N_CORES          # 64000 edges per core
NB = 2560                        # edges per gather batch (multiple of 256)
NBATCH = ESH // NB               # 25
GC = 896                         # idxs per dma_gather call (desc-ring cap)
SUP = 512                        # edges per supertile (= PSUM bank)
RANKS = N_ATOM // P              # 250 table stripes
LN_EPS = 1e-5

F32 = mybir.dt.float32
BF16 = mybir.dt.bfloat16
I16 = mybir.dt.int16
AF = mybir.ActivationFunctionType
ALU = mybir.AluOpType

_CACHE = {}


def _build(trivial_affine: bool, zero_bias: bool, ablate: frozenset = frozenset()):
    import os
    if not ablate and os.environ.get("K_ABLATE"):
        ablate = frozenset(os.environ["K_ABLATE"].split(","))
    nc = bacc.Bacc("TRN2", target_bir_lowering=False, debug=False,
                   enable_asserts=False, num_devices=N_CORES,
                   dynamic_dma_scratch_size=32768, num_swdge_queues=2)

    atomT_d = nc.dram_tensor("atomT", [2, P, N_ATOM], BF16, kind="ExternalInput")
    efT_d = nc.dram_tensor("efT", [P, ESH], BF16, kind="ExternalInput")
    idxd_d = nc.dram_tensor("idx_dst", [P, ESH // 16], I16, kind="ExternalInput")
    idxs_d = nc.dram_tensor("idx_src", [P, ESH // 16], I16, kind="ExternalInput")
    wl_d = nc.dram_tensor("wl16", [2, P, H], BF16, kind="ExternalInput")
    w1_d = nc.dram_tensor("w1t16", [3, P, H], BF16, kind="ExternalInput")
    w2_d = nc.dram_tensor("w2t16", [P, H], BF16, kind="ExternalInput")
    w3_d = nc.dram_tensor("w3t16", [P, H], BF16, kind="ExternalInput")
    if not zero_bias:
        b1_d = nc.dram_tensor("b1", [P, 1], F32, kind="ExternalInput")
        b2_d = nc.dram_tensor("b2", [P, 1], F32, kind="ExternalInput")
        b3_d = nc.dram_tensor("b3", [P, 1], F32, kind="ExternalInput")
    if not trivial_affine:
        gam_d = nc.dram_tensor("gam", [P, H], F32, kind="ExternalInput")
        bet_d = nc.dram_tensor("bet", [P, H], F32, kind="ExternalInput")
    out_d = nc.dram_tensor("out", [ESH, H], BF16, kind="ExternalOutput")

    with tile.TileContext(nc) as tc:
        with tc.tile_pool(name="const", bufs=1) as const:
            # --- constants ---------------------------------------------------
            wl0 = const.tile([P, H], BF16)
            nc.sync.dma_start(out=wl0[:], in_=wl_d[0])
            wl1 = const.tile([P, H], BF16)
            nc.sync.dma_start(out=wl1[:], in_=wl_d[1])
            w1a = const.tile([P, H], BF16)
            nc.sync.dma_start(out=w1a[:], in_=w1_d[0])
            w1b = const.tile([P, H], BF16)
            nc.sync.dma_start(out=w1b[:], in_=w1_d[1])
            w1c = const.tile([P, H], BF16)
            nc.sync.dma_start(out=w1c[:], in_=w1_d[2])
            w2 = const.tile([P, H], BF16)
            nc.sync.dma_start(out=w2[:], in_=w2_d[:])
            w3 = const.tile([P, H], BF16)
            nc.sync.dma_start(out=w3[:], in_=w3_d[:])
            if not zero_bias:
                b1 = const.tile([P, 1], F32)
                nc.sync.dma_start(out=b1[:], in_=b1_d[:])
                b2 = const.tile([P, 1], F32)
                nc.sync.dma_start(out=b2[:], in_=b2_d[:])
                b3 = const.tile([P, 1], F32)
                nc.sync.dma_start(out=b3[:], in_=b3_d[:])
            else:
                b1 = b2 = b3 = None
            if not trivial_affine:
                gam = const.tile([P, H], F32)
                nc.sync.dma_start(out=gam[:], in_=gam_d[:])
                bet = const.tile([P, H], F32)
                nc.sync.dma_start(out=bet[:], in_=bet_d[:])
            else:
                gam = bet = None
            ident = const.tile([P, P], BF16)
            make_identity(nc, ident[:])
            eps_t = const.tile([P, 1], F32)
            nc.vector.memset(eps_t[:], LN_EPS)
            idxd = const.tile([P, ESH // 16], I16)
            nc.sync.dma_start(out=idxd[:], in_=idxd_d[:])
            idxs = const.tile([P, ESH // 16], I16)
            nc.sync.dma_start(out=idxs[:], in_=idxs_d[:])
            # atom table, rank-stripe layout: atom a -> partition a%128,
            # free bytes [ (a//128)*256, +256 ) as 128 bf16 features.
            table = const.tile([P, RANKS, H], BF16)        # 62.5KB/partition

            # --- table build: s = A @ W_lin, written block-transposed -------
            # stationary lhsT = atomT half-block [128 infeat, 128 atoms]
            # moving rhs = W_lin half [128 infeat, 128 outfeat]
            # out = [128 atoms, 128 feats] = exactly one table stripe.
            ACH = 6400                                     # atoms per dma chunk
            assert N_ATOM % ACH == 0 and ACH % P == 0
            with tc.tile_pool(name="bld", bufs=2) as bld, \
                 tc.tile_pool(name="bldps", bufs=4, space="PSUM") as bldps:
                for ci in range(N_ATOM // ACH):
                    off = ci * ACH
                    a0 = bld.tile([P, ACH], BF16, tag="a0")
                    nc.sync.dma_start(out=a0[:], in_=atomT_d[0, :, off:off + ACH])
                    a1 = bld.tile([P, ACH], BF16, tag="a1")
                    nc.sync.dma_start(out=a1[:], in_=atomT_d[1, :, off:off + ACH])
                    for g in range((ACH + 511) // 512):    # 4 blocks per psum bank
                        nblk = min(4, (ACH - g * 512) // P)
                        ps = bldps.tile([P, 4, P], F32, space="PSUM", tag="bps")
                        for b in range(nblk):
                            s = g * 512 + b * P
                            nc.tensor.matmul(out=ps[:, b], lhsT=a0[:, s:s + P],
                                             rhs=wl0[:], start=True, stop=False)
                            nc.tensor.matmul(out=ps[:, b], lhsT=a1[:, s:s + P],
                                             rhs=wl1[:], start=False, stop=True)
                        st = (off + g * 512) // P          # first stripe idx
                        nc.vector.tensor_copy(
                            table[:, st:st + nblk].rearrange("p r f -> p (r f)"),
                            ps[:, :nblk].rearrange("p b f -> p (b f)"))

            # --- main loop ---------------------------------------------------
            tiles = [(i * SUP, SUP) for i in range(NB // SUP)]
            if NB % SUP:
                tiles.append(((NB // SUP) * SUP, NB % SUP))
            with tc.tile_pool(name="gat", bufs=2) as gat, \
                 tc.tile_pool(name="io", bufs=2) as io, \
                 tc.tile_pool(name="mid", bufs=4) as mid, \
                 tc.tile_pool(name="stat", bufs=8) as stat, \
                 tc.tile_pool(name="pmm", bufs=5, space="PSUM") as pmm, \
                 tc.tile_pool(name="ptr", bufs=3, space="PSUM") as ptr:
                def emit_batch(bi):
                    """Issue gathers + loads for batch bi; returns refs."""
                    e0 = bi * NB
                    i0 = bi * (NB // 16)
                    gd = gat.tile([P, 1, NB], BF16, tag="gd")
                    gs = gat.tile([P, 1, NB], BF16, tag="gs")
                    if "gather" in ablate:
                        nc.vector.memset(gd[:], 0.25)
                        nc.vector.memset(gs[:], 0.25)
                    else:
                        # chunk: the SWDGE descriptor ring (16KB/16=1024
                        # descs) caps a single dma_gather; >512 idxs hangs
                        # the exec unit on HW.
                        for q, (g, idxt) in enumerate(((gd, idxd), (gs, idxs))):
                            for off in range(0, NB, GC):
                                n = min(GC, NB - off)
                                nc.gpsimd.dma_gather(
                                    g[:, :, off:off + n],
                                    table[:].rearrange("p r f -> p (r f)"),
                                    idxt[:, i0 + off // 16:i0 + (off + n) // 16],
                                    n, n, H,
                                    transpose=True, sbuf_tokens_per_rank=P,
                                    sbuf_free_dim_per_rank=2 * H,
                                    queue_num=q)
                    eft = io.tile([P, NB], BF16, tag="eft")
                    nc.sync.dma_start(out=eft[:], in_=efT_d[:, e0:e0 + NB])
                    outb = io.tile([P, NB // 256, 2, P], BF16, tag="outb")
                    return dict(gd=gd, gs=gs, eft=eft, outb=outb, e0=e0)

                def mm_stage(B, toff, tw):
                    """L1..L3 matmuls + relus -> ps3 (PSUM)."""
                    ps1 = pmm.tile([P, SUP], F32, space="PSUM", tag="mm")
                    nc.tensor.matmul(out=ps1[:, :tw], lhsT=w1a[:],
                                     rhs=B["gd"][:, 0, toff:toff + tw],
                                     start=True, stop=False)
                    nc.tensor.matmul(out=ps1[:, :tw], lhsT=w1b[:],
                                     rhs=B["gs"][:, 0, toff:toff + tw],
                                     start=False, stop=False)
                    nc.tensor.matmul(out=ps1[:, :tw], lhsT=w1c[:],
                                     rhs=B["eft"][:, toff:toff + tw],
                                     start=False, stop=True)
                    h1 = mid.tile([P, SUP], BF16, tag="h1")
                    nc.scalar.activation(h1[:, :tw], ps1[:, :tw], AF.Relu,
                                         bias=b1[:, 0:1] if b1 else 0.0)
                    ps2 = pmm.tile([P, SUP], F32, space="PSUM", tag="mm")
                    nc.tensor.matmul(out=ps2[:, :tw], lhsT=w2[:],
                                     rhs=h1[:, :tw], start=True, stop=True)
                    h2 = mid.tile([P, SUP], BF16, tag="h2")
                    nc.scalar.activation(h2[:, :tw], ps2[:, :tw], AF.Relu,
                                         bias=b2[:, 0:1] if b2 else 0.0)
                    ps3 = pmm.tile([P, SUP], F32, space="PSUM", tag="mm")
                    nc.tensor.matmul(out=ps3[:, :tw], lhsT=w3[:],
                                     rhs=h2[:, :tw], start=True, stop=True)
                    return ps3

                def xt_stage(B, toff, tw, ps3):
                    """residual in T layout: xT = (ps3 + b3) + efT  (DVE)."""
                    xT = mid.tile([P, SUP], BF16, tag="xT")
                    if zero_bias:
                        nc.vector.tensor_tensor(
                            out=xT[:, :tw], in0=ps3[:, :tw],
                            in1=B["eft"][:, toff:toff + tw], op=ALU.add)
                    else:
                        h3s = mid.tile([P, SUP], BF16, tag="h3s")
                        nc.scalar.activation(h3s[:, :tw], ps3[:, :tw],
                                             AF.Identity, bias=b3[:, 0:1])
                        nc.vector.tensor_tensor(
                            out=xT[:, :tw], in0=h3s[:, :tw],
                            in1=B["eft"][:, toff:toff + tw], op=ALU.add)
                    return xT

                def tr_stage(tw, xT):
                    """transpose back (PE): block c = edges base+2p+(c%2)."""
                    nch = tw // P
                    xps = ptr.tile([P, 4, P], BF16, space="PSUM", tag="tr")
                    for c in range(nch):
                        nc.tensor.transpose(xps[:, c], xT[:, c * P:(c + 1) * P],
                                            ident[:])
                    return xps

                def ln_stage(B, toff, tw, xps):
                    """LN stats + normalize -> outb (DVE/Act)."""
                    nch = tw // P
                    outb = B["outb"]
                    bn = stat.tile([P, 4, 6], F32, tag="bn")
                    mv = stat.tile([P, 4, 2], F32, tag="mv")
                    for c in range(nch):
                        nc.vector.bn_stats(bn[:, c], xps[:, c])
                        nc.vector.bn_aggr(mv[:, c], bn[:, c])
                    std = stat.tile([P, 4], F32, tag="std")
                    nc.scalar.activation(std[:, :nch], mv[:, :nch, 1],
                                         AF.Sqrt, bias=eps_t[:, 0:1])
                    rstd = stat.tile([P, 4], F32, tag="rstd")
                    nc.vector.reciprocal(rstd[:, :nch], std[:, :nch])
                    nmr = stat.tile([P, 4], F32, tag="nmr")
                    nc.vector.tensor_tensor(out=nmr[:, :nch],
                                            in0=mv[:, :nch, 0],
                                            in1=rstd[:, :nch], op=ALU.mult)
                    nc.vector.tensor_scalar(out=nmr[:, :nch],
                                            in0=nmr[:, :nch], scalar1=-1.0,
                                            scalar2=None, op0=ALU.mult)
                    # apply: out = x*rstd + (-mean*rstd)  [+ affine]
                    for c in range(nch):
                        kk = toff // 256 + c // 2
                        j = c % 2
                        if c != 0:
                            nc.scalar.activation(outb[:, kk, j], xps[:, c],
                                                 AF.Identity,
                                                 bias=nmr[:, c:c + 1],
                                                 scale=rstd[:, c:c + 1])
                        else:
                            nc.vector.tensor_scalar(
                                out=outb[:, kk, j], in0=xps[:, c],
                                scalar1=rstd[:, c:c + 1],
                                scalar2=nmr[:, c:c + 1],
                                op0=ALU.mult, op1=ALU.add)
                        if not trivial_affine:
                            nc.vector.tensor_tensor(out=outb[:, kk, j],
                                                    in0=outb[:, kk, j],
                                                    in1=gam[:], op=ALU.mult)
                            nc.vector.tensor_tensor(out=outb[:, kk, j],
                                                    in0=outb[:, kk, j],
                                                    in1=bet[:], op=ALU.add)

                def out_stage(B):
                    nc.sync.dma_start(
                        out=out_d[B["e0"]:B["e0"] + NB, :].rearrange(
                            "(k p j) f -> p k (j f)", p=P, j=2),
                        in_=B["outb"][:])

                # two-deep software pipeline ordered so each engine's
                # in-order stream is ready-when-reached:
                #   mm(t) [PE/Act] ; xT(t-1) [DVE] ; tr(t-1) [PE] ;
                #   ln(t-2) [DVE/Act]
                work = [(bi, toff, tw) for bi in range(NBATCH)
                        for (toff, tw) in tiles]
                batches = {}
                s1 = s2 = None          # (bi, toff, tw, ps3) / (.., xps)
                ln_done = collections.Counter()
                ntiles = len(tiles)

                def run_ln(item):
                    bi, toff, tw, xps = item
                    ln_stage(batches[bi], toff, tw, xps)
                    ln_done[bi] += 1
                    if ln_done[bi] == ntiles:
                        out_stage(batches[bi])

                if "mlp" in ablate:
                    for bi in range(NBATCH):
                        B = emit_batch(bi)
                        nc.vector.tensor_copy(
                            B["outb"][:].rearrange("p k j f -> p (k j f)"),
                            B["gd"][:, 0].bitcast(BF16)[:, :NB])
                        out_stage(B)
                else:
                    for (bi, toff, tw) in work:
                        if bi not in batches:
                            batches[bi] = emit_batch(bi)
                        ps3 = mm_stage(batches[bi], toff, tw)
                        if s1 is not None:
                            b1_, t1, w1_, p3 = s1
                            xT = xt_stage(batches[b1_], t1, w1_, p3)
                            xps = tr_stage(w1_, xT)
                            if s2 is not None:
                                run_ln(s2)
                            s2 = (b1_, t1, w1_, xps)
                        s1 = (bi, toff, tw, ps3)
                    # drain
                    b1_, t1, w1_, p3 = s1
                    xT = xt_stage(batches[b1_], t1, w1_, p3)
                    xps = tr_stage(w1_, xT)
                    if s2 is not None:
                        run_ln(s2)
                    run_ln((b1_, t1, w1_, xps))

    nc.compile()
    return nc


def _make_runner(nc):
    """shard_map runner over 8 cores (no donation so it can be re-invoked)."""
    import jax
    from jax.sharding import Mesh, PartitionSpec
    from jax.experimental.shard_map import shard_map
    from concourse import bass2jax

    bass2jax.install_neuronx_cc_hook()

    partition_name = (nc.partition_id_tensor.name
                      if nc.partition_id_tensor else None)
    in_names, out_names, out_avals, zero_shapes = [], [], [], []
    for alloc in nc.m.functions[0].allocations:
        if not isinstance(alloc, mybir.MemoryLocationSet):
            continue
        name = alloc.memorylocations[0].name
        if alloc.kind == "ExternalInput":
            if name != partition_name:
                in_names.append(name)
        elif alloc.kind == "ExternalOutput":
            out_names.append(name)
            out_avals.append(jax.core.ShapedArray(
                tuple(alloc.tensor_shape), mybir.dt.np(alloc.dtype)))
            zero_shapes.append((tuple(alloc.tensor_shape), mybir.dt.np(alloc.dtype)))
    n_params = len(in_names)
    all_in_names = in_names + out_names
    if partition_name is not None:
        all_in_names = all_in_names + [partition_name]

    def _body(*args):
        operands = list(args)
        if partition_name is not None:
            operands.append(bass2jax.partition_id_tensor())
        outs = bass2jax._bass_exec_p.bind(
            *operands,
            out_avals=tuple(out_avals),
            in_names=tuple(all_in_names),
            out_names=tuple(out_names),
            lowering_input_output_aliases=(),
            sim_require_finite=True,
            sim_require_nnan=True,
            nc=nc,
        )
        return tuple(outs)

    devices = jax.devices()[:N_CORES]
    mesh = Mesh(np.asarray(devices), ("core",))
    nin = n_params + len(out_names)
    sharded = jax.jit(
        shard_map(_body, mesh=mesh,
                  in_specs=(PartitionSpec("core"),) * nin,
                  out_specs=(PartitionSpec("core"),) * len(out_names),
                  check_rep=False),
        keep_unused=True)
    return sharded, in_names, out_names, zero_shapes


def _pair_permute_idx(idx_flat: np.ndarray) -> np.ndarray:
    """Reorder edges within each 256-chunk to [evens, odds]."""
    return idx_flat.reshape(-1, P, 2).transpose(0, 2, 1).reshape(-1)


def _wrap_idx(idx_flat: np.ndarray) -> np.ndarray:
    """ap/dma_gather wrapped-index layout: idx[p, s] covers edge s*16 + p%16,
    replicated across the 8 gpsimd 16-partition core groups."""
    a = idx_flat.astype(np.int16).reshape(-1, 16).T        # [16, esh//16]
    return np.tile(a, (8, 1))                              # [128, esh//16]


def _prep(inputs):
    atom_features = np.asarray(inputs["atom_features"], dtype=np.float32)
    edge_features = np.asarray(inputs["edge_features"], dtype=np.float32)
    edge_index = np.asarray(inputs["edge_index"]).astype(np.int64)
    wlin = np.asarray(inputs["W_lin"], dtype=np.float32)
    w1 = np.asarray(inputs["W1"], dtype=np.float32)
    w2 = np.asarray(inputs["W2"], dtype=np.float32)
    w3 = np.asarray(inputs["W3"], dtype=np.float32)
    b1 = np.asarray(inputs["b1"], dtype=np.float32)
    b2 = np.asarray(inputs["b2"], dtype=np.float32)
    b3 = np.asarray(inputs["b3"], dtype=np.float32)
    gamma = np.asarray(inputs["gamma"], dtype=np.float32)
    beta = np.asarray(inputs["beta"], dtype=np.float32)

    trivial_affine = bool(np.all(gamma == 1.0) and np.all(beta == 0.0))
    zero_bias = bool(np.all(b1 == 0.0) and np.all(b2 == 0.0) and np.all(b3 == 0.0))

    bf = ml_dtypes.bfloat16
    at = atom_features.T.astype(bf)                        # [256, 32000]
    atomT = np.stack([at[:P], at[P:]])                     # [2, 128, 32000]

    shared = {
        "atomT": atomT,
        "wl16": wlin.astype(bf).reshape(2, P, H),
        "w1t16": w1.astype(bf).reshape(3, P, H),
        "w2t16": w2.astype(bf),
        "w3t16": w3.astype(bf),
    }
    if not zero_bias:
        shared["b1"] = b1.reshape(P, 1)
        shared["b2"] = b2.reshape(P, 1)
        shared["b3"] = b3.reshape(P, 1)
    if not trivial_affine:
        shared["gam"] = np.tile(gamma.reshape(1, H), (P, 1)).astype(np.float32)
        shared["bet"] = np.tile(beta.reshape(1, H), (P, 1)).astype(np.float32)

    in_maps = []
    for c in range(N_CORES):
        e0 = c * ESH
        ef = edge_features[e0:e0 + ESH]
        # pair-permute: within each 256-edge chunk -> [evens, odds]
        efp = ef.reshape(-1, P, 2, H).transpose(0, 2, 1, 3).reshape(ESH, H)
        m = dict(shared)
        m["efT"] = efp.T.astype(bf)                        # [128, 64000]
        m["idx_dst"] = _wrap_idx(_pair_permute_idx(edge_index[0, e0:e0 + ESH]))
        m["idx_src"] = _wrap_idx(_pair_permute_idx(edge_index[1, e0:e0 + ESH]))
        in_maps.append(m)
    return in_maps, trivial_affine, zero_bias


def _get_compiled(trivial_affine: bool, zero_bias: bool):
    key = ("k2", trivial_affine, zero_bias)
    if key not in _CACHE:
        nc = _build(trivial_affine, zero_bias)
        runner = _make_runner(nc)
        _CACHE[key] = (nc, runner)
    return _CACHE[key]


def _concat_inputs(in_maps, in_names, zero_shapes):
    concat_in = [
        np.concatenate([np.asarray(in_maps[c][n]) for c in range(N_CORES)], axis=0)
        for n in in_names
    ]
    concat_zero = [
        np.zeros((N_CORES * s[0], *s[1:]), dt) for (s, dt) in zero_shapes
    ]
    return concat_in, concat_zero


def kernel(**inputs) -> np.ndarray:
    in_maps, trivial_affine, zero_bias = _prep(inputs)
    _, (sharded, in_names, out_names, zero_shapes) = _get_compiled(
        trivial_affine, zero_bias)
    concat_in, concat_zero = _concat_inputs(in_maps, in_names, zero_shapes)
    outs = sharded(*concat_in, *concat_zero)
    oi = out_names.index("out")
    full = np.asarray(outs[oi]).reshape(N_CORES * ESH, H)
    return full.astype(np.float32)


def bench(inputs, reps: int = 10):
    """Returns (exec_times_seconds, results) using device-resident inputs."""
    import jax, time
    from jax.sharding import Mesh, PartitionSpec, NamedSharding
    in_maps, trivial_affine, zero_bias = _prep(inputs)
    _, (sharded, in_names, out_names, zero_shapes) = _get_compiled(
        trivial_affine, zero_bias)
    concat_in, concat_zero = _concat_inputs(in_maps, in_names, zero_shapes)
    devices = jax.devices()[:N_CORES]
    mesh = Mesh(np.asarray(devices), ("core",))
    sh = NamedSharding(mesh, PartitionSpec("core"))
    args = [jax.device_put(a, sh) for a in concat_in + concat_zero]
    outs = sharded(*args)  # warm-up + compile
    jax.block_until_ready(outs)
    times = []
    for _ in range(reps):
        t0 = time.perf_counter()
        outs = sharded(*args)
        jax.block_until_ready(outs)
        times.append(time.perf_counter() - t0)
    # pipelined dispatch: amortizes per-call host/tunnel overhead
    npipe = 30
    t0 = time.perf_counter()
    for _ in range(npipe):
        outs = sharded(*args)
    jax.block_until_ready(outs)
    pipe_per_call = (time.perf_counter() - t0) / npipe
    times.append(pipe_per_call)
    oi = out_names.index("out")
    full = np.asarray(outs[oi]).reshape(N_CORES * ESH, H).astype(np.float32)
    return times, full
